# revision 5
# baseline (speedup 1.0000x reference)
"""Trainium2 kernel for the bilinear form y[b,k] = sum_ij x[b,i] x[b,j] W[i,j,k] + b[k].

Shapes: x (512, 784) f32, W (614656=784*784, 10) f32, b (10,) f32 -> y (512, 10) f32.

Strategy (8 NeuronCores):
  - Shard the j axis of W.reshape(784, 784, 10) across cores: 98 j's per core.
    Each core reads W/8 (~3.07 MB) + full x (~1.6 MB); DMA ~= PE time (ridge).
  - Stage 1 (TensorE): U[b, (k,j)] = sum_i x[b,i] * W[i, j_shard, k], with the
    x^T tile stationary and the W shard as the moving operand, accumulating
    over 7 i-tiles in PSUM.
  - Stage 2 (VectorE): fused multiply+reduce over j:
    y_part[b, k] = sum_j U[b, (k,j)] * x[b, j_shard_j].
  - Host: y = sum_c y_part_c + b  (20 KB per core; no collectives needed).
"""

import numpy as np

D = 784
B = 512
C = 10
NCORES = 8
JS = D // NCORES  # 98 j's per core
JK = JS * C  # 980 free columns per core, laid out as (k, j)
HALF = JK // 2  # 490 = 5 k's x 98 j's -> one PSUM bank
P = 128
B_TILES = B // P  # 4
I_TILES = [(i, min(P, D - i)) for i in range(0, D, P)]  # 6x128 + 1x16

MM_DTYPE = "float32r"  # reduced-precision fp32 matmul: 1 cycle/row at N>=256

_nc_cache = {}


def _build_nc():
    import concourse.bacc as bacc
    import concourse.mybir as mybir
    import concourse.tile as tile

    mm_dt = getattr(mybir.dt, MM_DTYPE)

    nc = bacc.Bacc("TRN2", target_bir_lowering=False)

    xT = nc.dram_tensor("xT", [D, B], mm_dt, kind="ExternalInput")
    w = nc.dram_tensor("w", [D, JK], mm_dt, kind="ExternalInput")
    xs = nc.dram_tensor("xs", [B, JS], mybir.dt.float32, kind="ExternalInput")
    y = nc.dram_tensor("y", [B, C], mybir.dt.float32, kind="ExternalOutput")

    with tile.TileContext(nc) as tc:
        with (
            tc.tile_pool(name="wpool", bufs=len(I_TILES)) as wpool,
            tc.tile_pool(name="xpool", bufs=len(I_TILES)) as xpool,
            tc.tile_pool(name="xspool", bufs=B_TILES) as xspool,
            tc.tile_pool(name="ypool", bufs=B_TILES) as ypool,
            tc.tile_pool(name="scratch", bufs=2) as spool,
            tc.tile_pool(name="psum", bufs=4, space="PSUM") as psum_pool,
        ):
            w_sb = []
            xT_sb = []
            for i0, isz in I_TILES:
                wt = wpool.tile([isz, JK], mm_dt)
                nc.sync.dma_start(wt[:], w[i0 : i0 + isz, :])
                w_sb.append(wt)
                xt = xpool.tile([isz, B], mm_dt)
                nc.sync.dma_start(xt[:], xT[i0 : i0 + isz, :])
                xT_sb.append(xt)
            xs_sb = []
            for bt in range(B_TILES):
                xst = xspool.tile([P, JS], mybir.dt.float32)
                nc.sync.dma_start(xst[:], xs[bt * P : (bt + 1) * P, :])
                xs_sb.append(xst)

            for bt in range(B_TILES):
                y_t = ypool.tile([P, C], mybir.dt.float32)
                for h in range(2):
                    pt = psum_pool.tile([P, HALF], mybir.dt.float32)
                    for it, (i0, isz) in enumerate(I_TILES):
                        nc.tensor.matmul(
                            pt[:],
                            xT_sb[it][:, bt * P : (bt + 1) * P],
                            w_sb[it][:, h * HALF : (h + 1) * HALF],
                            start=(it == 0),
                            stop=(it == len(I_TILES) - 1),
                        )
                    scr = spool.tile([P, HALF], mybir.dt.float32)
                    pt3 = pt[:].rearrange("p (k j) -> p k j", k=C // 2)
                    scr3 = scr[:].rearrange("p (k j) -> p k j", k=C // 2)
                    xs3 = xs_sb[bt][:, None, :].broadcast_to([P, C // 2, JS])
                    nc.vector.tensor_tensor(scr3, pt3, xs3, mybir.AluOpType.mult)
                    nc.vector.tensor_reduce(
                        out=y_t[:, h * 5 : (h + 1) * 5],
                        in_=scr3,
                        op=mybir.AluOpType.add,
                        axis=mybir.AxisListType.X,
                    )
                nc.sync.dma_start(y[bt * P : (bt + 1) * P, :], y_t[:])

    nc.compile()
    return nc


def _get_nc():
    if "nc" not in _nc_cache:
        _nc_cache["nc"] = _build_nc()
    return _nc_cache["nc"]


def _make_in_maps(x, W):
    Wr = np.asarray(W, dtype=np.float32).reshape(D, D, C)
    xT = np.ascontiguousarray(np.asarray(x, dtype=np.float32).T)
    in_maps = []
    for c in range(NCORES):
        js, je = c * JS, (c + 1) * JS
        # w_shard[i, k*JS + j] = W[i, js+j, k]
        wshard = np.ascontiguousarray(
            Wr[:, js:je, :].transpose(0, 2, 1).reshape(D, JK)
        )
        xsl = np.ascontiguousarray(np.asarray(x, dtype=np.float32)[:, js:je])
        in_maps.append({"xT": xT, "w": wshard, "xs": xsl})
    return in_maps


def run_spmd(x, W, **spmd_kwargs):
    """Compile/run the SPMD kernel; returns (partials, BassKernelResults)."""
    from concourse.bass_utils import run_bass_kernel_spmd

    nc = _get_nc()
    in_maps = _make_in_maps(x, W)
    res = run_bass_kernel_spmd(nc, in_maps, core_ids=list(range(NCORES)), **spmd_kwargs)
    return [r["y"] for r in res.results], res


def kernel(x, W, b):
    partials, _ = run_spmd(x, W)
    y = np.sum(np.stack(partials, 0), axis=0, dtype=np.float64) + np.asarray(
        b, dtype=np.float64
    )
    return y.astype(np.float32)


# revision 7
# speedup vs baseline: 1.2537x; 1.2537x over previous
"""Trainium2 kernel for the bilinear form y[b,k] = sum_ij x[b,i] x[b,j] W[i,j,k] + b[k].

Shapes: x (512, 784) f32, W (614656=784*784, 10) f32, b (10,) f32 -> y (512, 10) f32.

Strategy (8 NeuronCores):
  - Shard the j axis of W.reshape(784, 784, 10) across cores: 98 j's per core.
    Each core reads W/8 + full x; DMA ~= PE time (ridge regime).
  - Stage 1 (TensorE): U[b, (k,j)] = sum_i x[b,i] * W[i, j_shard, k], with the
    x^T tile stationary and the W shard as the moving operand, accumulating
    over 7 i-tiles in PSUM (fp32 accumulation).
  - Stage 2 (VectorE): multiply by x[b, j_shard] (broadcast over k) + reduce
    over j: y_part[b, k] = sum_j U[b, (k,j)] * x[b, j].
  - Host: y = sum_c y_part_c + b  (20 KB per core; no collectives needed).

Matmul operands are fp16 (PE runs 1 cycle/row; halves DMA traffic). The
contraction is only 784 terms with fp32 PSUM accumulation, so the fp16
input rounding gives ~1e-3 relative error on y (measured 7e-4).
"""

import numpy as np

D = 784
B = 512
C = 10
NCORES = 8
JS = D // NCORES  # 98 j's per core
JK = JS * C  # 980 free columns per core, laid out as (k, j)
HALF = JK // 2  # 490 = 5 k's x 98 j's -> one PSUM bank
P = 128
B_TILES = B // P  # 4
I_TILES = [(i, min(P, D - i)) for i in range(0, D, P)]  # 6x128 + 1x16

MM_DTYPE = "float16"  # dtype of the matmul operands (and their DMA)

_nc_cache = {}


def _build_nc():
    import concourse.bacc as bacc
    import concourse.mybir as mybir
    import concourse.tile as tile

    mm_dt = getattr(mybir.dt, MM_DTYPE)

    nc = bacc.Bacc("TRN2", target_bir_lowering=False)

    xT = nc.dram_tensor("xT", [D, B], mm_dt, kind="ExternalInput")
    w = nc.dram_tensor("w", [D, JK], mm_dt, kind="ExternalInput")
    xs = nc.dram_tensor("xs", [B, JS], mybir.dt.float32, kind="ExternalInput")
    y = nc.dram_tensor("y", [B, C], mybir.dt.float32, kind="ExternalOutput")

    with tile.TileContext(nc) as tc:
        with (
            tc.tile_pool(name="wpool", bufs=2 * len(I_TILES)) as wpool,
            tc.tile_pool(name="xpool", bufs=len(I_TILES)) as xpool,
            tc.tile_pool(name="xspool", bufs=B_TILES) as xspool,
            tc.tile_pool(name="ypool", bufs=B_TILES) as ypool,
            tc.tile_pool(name="scratch", bufs=3) as spool,
            tc.tile_pool(name="psum", bufs=6, space="PSUM") as psum_pool,
        ):
            # x^T tiles first (needed by every accumulation group).
            xT_sb = []
            for i0, isz in I_TILES:
                xt = xpool.tile([isz, B], mm_dt)
                nc.sync.dma_start(xt[:], xT[i0 : i0 + isz, :])
                xT_sb.append(xt)
            # W shard, half (h) major so the first PSUM group's 7 i-tiles
            # arrive before the second half's.
            w_sb = {}
            for h in range(2):
                for it, (i0, isz) in enumerate(I_TILES):
                    wt = wpool.tile([isz, HALF], mm_dt)
                    nc.sync.dma_start(
                        wt[:], w[i0 : i0 + isz, h * HALF : (h + 1) * HALF]
                    )
                    w_sb[(it, h)] = wt
            xs_sb = []
            for bt in range(B_TILES):
                xst = xspool.tile([P, JS], mybir.dt.float32)
                nc.sync.dma_start(xst[:], xs[bt * P : (bt + 1) * P, :])
                xs_sb.append(xst)

            y_ts = [
                ypool.tile([P, C], mybir.dt.float32, name=f"y_t{bt}")
                for bt in range(B_TILES)
            ]
            for h in range(2):
                for bt in range(B_TILES):
                    y_t = y_ts[bt]
                    pt = psum_pool.tile([P, HALF], mybir.dt.float32)
                    for it, (i0, isz) in enumerate(I_TILES):
                        nc.tensor.matmul(
                            pt[:],
                            xT_sb[it][:, bt * P : (bt + 1) * P],
                            w_sb[(it, h)][:],
                            start=(it == 0),
                            stop=(it == len(I_TILES) - 1),
                        )
                    scr = spool.tile([P, HALF], mybir.dt.float32)
                    pt3 = pt[:].rearrange("p (k j) -> p k j", k=C // 2)
                    scr3 = scr[:].rearrange("p (k j) -> p k j", k=C // 2)
                    xs3 = xs_sb[bt][:, None, :].broadcast_to([P, C // 2, JS])
                    nc.vector.tensor_tensor(scr3, pt3, xs3, mybir.AluOpType.mult)
                    nc.vector.tensor_reduce(
                        out=y_t[:, h * 5 : (h + 1) * 5],
                        in_=scr3,
                        op=mybir.AluOpType.add,
                        axis=mybir.AxisListType.X,
                    )
                    if h == 1:
                        nc.sync.dma_start(y[bt * P : (bt + 1) * P, :], y_t[:])

    nc.compile()
    return nc


def _get_nc():
    if "nc" not in _nc_cache:
        _nc_cache["nc"] = _build_nc()
    return _nc_cache["nc"]


def _make_in_maps(x, W):
    import concourse.mybir as mybir

    mm_np = mybir.dt.np(getattr(mybir.dt, MM_DTYPE))
    x = np.asarray(x, dtype=np.float32)
    Wr = np.asarray(W, dtype=np.float32).reshape(D, D, C)
    xT = np.ascontiguousarray(x.T.astype(mm_np))
    in_maps = []
    for c in range(NCORES):
        js, je = c * JS, (c + 1) * JS
        # w_shard[i, k*JS + j] = W[i, js+j, k]
        wshard = np.ascontiguousarray(
            Wr[:, js:je, :].transpose(0, 2, 1).reshape(D, JK).astype(mm_np)
        )
        xsl = np.ascontiguousarray(x[:, js:je])
        in_maps.append({"xT": xT, "w": wshard, "xs": xsl})
    return in_maps


def run_spmd(x, W, **spmd_kwargs):
    """Compile/run the SPMD kernel; returns (partials, BassKernelResults)."""
    from concourse.bass_utils import run_bass_kernel_spmd

    nc = _get_nc()
    in_maps = _make_in_maps(x, W)
    res = run_bass_kernel_spmd(nc, in_maps, core_ids=list(range(NCORES)), **spmd_kwargs)
    return [r["y"] for r in res.results], res


def kernel(x, W, b):
    partials, _ = run_spmd(x, W)
    y = np.sum(np.stack(partials, 0), axis=0, dtype=np.float64) + np.asarray(
        b, dtype=np.float64
    )
    return y.astype(np.float32)


# revision 9
# speedup vs baseline: 1.3749x; 1.0967x over previous
"""Trainium2 kernel for the bilinear form y[b,k] = sum_ij x[b,i] x[b,j] W[i,j,k] + b[k].

Shapes: x (512, 784) f32, W (614656=784*784, 10) f32, b (10,) f32 -> y (512, 10) f32.

Strategy (8 NeuronCores):
  - Shard the j axis of W.reshape(784, 784, 10) across cores: 98 j's per core.
    Each core reads W/8 + full x (~2.9 MB in fp16); DMA ~= PE time (ridge).
  - Stage 1 (TensorE): U[b, (k,j)] = sum_i x[b,i] * W[i, j_shard, k], x^T tiles
    stationary, W shard moving, accumulating over 7 uniform 112-row i-tiles
    in PSUM (fp32).
  - Stage 2 (VectorE): multiply by x[b, j_shard] (broadcast over k) and reduce
    over j: y_part[b, k] = sum_j U[b, (k,j)] * x[b, j].
  - Host: y = sum_c y_part_c + b  (20 KB per core; no collectives needed).

Perf notes:
  - Matmul operands are fp16: PE streams 1 column/cycle (fp32 is 4x slower,
    fp32r self-loads weights every matmul), and DMA traffic halves.
    fp32 PSUM accumulation keeps the overall error ~1e-3.
  - Host pre-arranges xT/w into partition-major layouts so each DMA moves
    2-7 KB contiguous per partition (near-peak HBM rate), issued as a few
    large transfers split between the two HWDGE rings (sync + scalar).
  - i-tiles stream in 3 chunks; the matmul loop is i-outer so the PE can
    start after the first chunk (~0.5 MB) instead of the whole shard.
"""

import numpy as np

D = 784
B = 512
C = 10
NCORES = 8
JS = D // NCORES  # 98 j's per core
JK = JS * C  # 980 free columns per core, laid out as (k, j)
HALF = JK // 2  # 490 = 5 k's x 98 j's -> one PSUM bank
P = 128
B_TILES = B // P  # 4
IT = 7  # i-tiles
IP = D // IT  # 112 rows per i-tile (uniform, no padding)
CHUNKS = [(0, 2), (2, 4), (4, 7)]  # i-tile chunks for DMA/compute pipelining

MM_DTYPE = "float16"  # dtype of the matmul operands (and their DMA)

_nc_cache = {}


def _build_nc():
    import concourse.bacc as bacc
    import concourse.mybir as mybir
    import concourse.tile as tile

    mm_dt = getattr(mybir.dt, MM_DTYPE)

    nc = bacc.Bacc("TRN2", target_bir_lowering=False)

    # Partition-major DRAM layouts (see _make_in_maps).
    xT = nc.dram_tensor("xT", [IP, IT, B], mm_dt, kind="ExternalInput")
    w = nc.dram_tensor("w", [2, IP, IT, HALF], mm_dt, kind="ExternalInput")
    xs = nc.dram_tensor("xs", [P, B_TILES, JS], mybir.dt.float32, kind="ExternalInput")
    y = nc.dram_tensor("y", [P, B_TILES, C], mybir.dt.float32, kind="ExternalOutput")

    with tile.TileContext(nc) as tc:
        with (
            tc.tile_pool(name="wpool", bufs=6) as wpool,
            tc.tile_pool(name="xpool", bufs=3) as xpool,
            tc.tile_pool(name="xspool", bufs=1) as xspool,
            tc.tile_pool(name="ypool", bufs=1) as ypool,
            tc.tile_pool(name="scratch", bufs=4) as spool,
            tc.tile_pool(name="psum", bufs=8, space="PSUM") as psum_pool,
        ):
            # x^T chunks on the scalar HWDGE ring; w chunks on the sync ring.
            xT_sb = {}
            for c0, c1 in CHUNKS:
                xt = xpool.tile([IP, c1 - c0, B], mm_dt, name=f"xt_c{c0}", tag="xt")
                nc.scalar.dma_start(xt[:], xT[:, c0:c1, :])
                for it in range(c0, c1):
                    xT_sb[it] = xt[:, it - c0, :]
            w_sb = {}
            for h in range(2):
                for c0, c1 in CHUNKS:
                    wt = wpool.tile([IP, c1 - c0, HALF], mm_dt, name=f"w_h{h}c{c0}", tag="w")
                    nc.sync.dma_start(wt[:], w[h, :, c0:c1, :])
                    for it in range(c0, c1):
                        w_sb[(it, h)] = wt[:, it - c0, :]
            xs_sb = xspool.tile([P, B_TILES, JS], mybir.dt.float32)
            nc.scalar.dma_start(xs_sb[:], xs[:])

            y_t = ypool.tile([P, B_TILES, C], mybir.dt.float32)
            pts = {}
            for h in range(2):
                for it in range(IT):
                    for bt in range(B_TILES):
                        if it == 0:
                            pts[bt] = psum_pool.tile(
                                [P, HALF],
                                mybir.dt.float32,
                                name=f"pt_h{h}b{bt}",
                                tag="pt",
                            )
                        nc.tensor.matmul(
                            pts[bt][:],
                            xT_sb[it][:, bt * P : (bt + 1) * P],
                            w_sb[(it, h)][:],
                            start=(it == 0),
                            stop=(it == IT - 1),
                        )
                for bt in range(B_TILES):
                    pt = pts[bt]
                    scr = spool.tile([P, HALF], mybir.dt.float32)
                    pt3 = pt[:].rearrange("p (k j) -> p k j", k=C // 2)
                    scr3 = scr[:].rearrange("p (k j) -> p k j", k=C // 2)
                    xs3 = xs_sb[:, bt, None, :].broadcast_to([P, C // 2, JS])
                    nc.vector.tensor_tensor(scr3, pt3, xs3, mybir.AluOpType.mult)
                    nc.vector.tensor_reduce(
                        out=y_t[:, bt, h * 5 : (h + 1) * 5],
                        in_=scr3,
                        op=mybir.AluOpType.add,
                        axis=mybir.AxisListType.X,
                    )
            nc.scalar.dma_start(y[:], y_t[:])

    nc.compile()
    return nc


def _get_nc():
    if "nc" not in _nc_cache:
        _nc_cache["nc"] = _build_nc()
    return _nc_cache["nc"]


def _make_in_maps(x, W):
    import concourse.mybir as mybir

    mm_np = mybir.dt.np(getattr(mybir.dt, MM_DTYPE))
    x = np.asarray(x, dtype=np.float32)
    Wr = np.asarray(W, dtype=np.float32).reshape(D, D, C)
    # xT_dram[p, t, b] = x[b, t*IP + p]
    xT = np.ascontiguousarray(
        x.T.astype(mm_np).reshape(IT, IP, B).transpose(1, 0, 2)
    )
    # xs_dram[p, t, j] = x[t*P + p, js + j]  (per-core slice below)
    in_maps = []
    for c in range(NCORES):
        js, je = c * JS, (c + 1) * JS
        # wsh[i, k*JS + j] = W[i, js+j, k]; then [h, p, t, col] partition-major
        wsh = Wr[:, js:je, :].transpose(0, 2, 1).reshape(D, JK).astype(mm_np)
        wshard = np.ascontiguousarray(
            wsh.reshape(IT, IP, 2, HALF).transpose(2, 1, 0, 3)
        )
        xsl = np.ascontiguousarray(
            x[:, js:je].reshape(B_TILES, P, JS).transpose(1, 0, 2)
        )
        in_maps.append({"xT": xT, "w": wshard, "xs": xsl})
    return in_maps


def run_spmd(x, W, **spmd_kwargs):
    """Compile/run the SPMD kernel; returns (partials, BassKernelResults)."""
    from concourse.bass_utils import run_bass_kernel_spmd

    nc = _get_nc()
    in_maps = _make_in_maps(x, W)
    res = run_bass_kernel_spmd(nc, in_maps, core_ids=list(range(NCORES)), **spmd_kwargs)
    # y_dram[p, t, k] -> y[t*P + p, k]
    partials = [
        r["y"].transpose(1, 0, 2).reshape(B, C) for r in res.results
    ]
    return partials, res


def kernel(x, W, b):
    partials, _ = run_spmd(x, W)
    y = np.sum(np.stack(partials, 0), axis=0, dtype=np.float64) + np.asarray(
        b, dtype=np.float64
    )
    return y.astype(np.float32)
